# revision 8
# baseline (speedup 1.0000x reference)
"""Trainium2 Bass kernel for the SG-visibility sampling network.

Math notes (exploited structure):
  - U,V are orthogonal to the unit lobe axis l, so dot(sample_dir, l) == cos(r_phi)
    exactly (up to fp eps).  Hence the SG weight w = exp(sharp*(cos_phi-1)) is a
    per-lobe constant and sum_s(vis*w)/(sum_s w + TINY) = scale_l * sum_s vis with
    scale_l = w/(S*w + TINY), precomputed on host.
  - pre-activation of the hidden layer decomposes as
        pre_h[n,l,s,h] = P_n[h] - C_l[h] - ct[n,l,s]*A_l[h] - st[n,l,s]*B_l[h]
    with P_n = p_n @ W1[:3] + b1,  A_l = sp_l*(U_l@Wd),  B_l = sp_l*(V_l@Wd),
    C_l = cp_l*(l_l@Wd),  Wd = root_rot @ W1[3:].
  - hemisphere mask: cos_term = ct*a_nl + st*b_nl + c_nl with
    a = normals@(sp*U)_l, b = normals@(sp*V)_l, c = normals@(cp*l)_l.

Device layout (per core, data-parallel over N):  partitions = (lobe-in-chunk 16,
s 8) = 128, free dim = rays n (1024).  Per (lobe, n-half) the 8x16=(s,h) hidden
pre-activation lives in PSUM as (s,h) x n, built from 3 accumulating matmuls
with zero-padded K=128 stationary weights (PE base-partition rule: operands
must start at partition 0/32/64, so per-lobe K-slices are packed into lhsT
zeros instead).
"""

import numpy as np

N, L, S, H = 8192, 128, 8, 16
NCORES = 8
NC = N // NCORES          # rays per core
LPC = 16                  # lobes per chunk
CHUNKS = L // LPC
TINY = 1e-6

_PROG = None


def _build_program():
    import concourse.bass as bass
    import concourse.bacc as bacc
    import concourse.mybir as mybir
    import concourse.tile as tile

    f32 = mybir.dt.float32
    AF = mybir.ActivationFunctionType
    ALU = mybir.AluOpType
    PI4 = float(np.pi / 4.0)

    nc = bacc.Bacc("TRN2", target_bir_lowering=False, debug=False,
                   num_devices=NCORES)

    rt = nc.declare_dram_parameter("rt", [L * S, NC], f32, isOutput=False)
    nrmT = nc.declare_dram_parameter("nrmT", [3, NC], f32, isOutput=False)
    pc = nc.declare_dram_parameter("pc", [4, NC], f32, isOutput=False)
    wct = nc.declare_dram_parameter("wct", [128, L * 128], f32, isOutput=False)
    wst = nc.declare_dram_parameter("wst", [128, L * 128], f32, isOutput=False)
    wpc = nc.declare_dram_parameter("wpc", [4, L * 128], f32, isOutput=False)
    wabc = nc.declare_dram_parameter("wabc", [3, 3 * L * 8], f32, isOutput=False)
    wsig = nc.declare_dram_parameter("wsig", [128, 512], f32, isOutput=False)
    wsum = nc.declare_dram_parameter("wsum", [128, CHUNKS * L], f32, isOutput=False)
    cb = nc.declare_dram_parameter("cb", [128, 4], f32, isOutput=False)
    out = nc.declare_dram_parameter("out", [L, NC], f32, isOutput=True)

    HF = NC // 2  # matmul moving-operand free-dim limit for fp32

    with tile.TileContext(nc) as tc:
        with (
            tc.tile_pool(name="const", bufs=1) as cpool,
            tc.tile_pool(name="io", bufs=2) as io,
            tc.tile_pool(name="wp", bufs=2) as wpool,
            tc.tile_pool(name="trig", bufs=2) as trig,
            tc.tile_pool(name="work", bufs=2) as work,
            tc.tile_pool(name="hrp", bufs=3) as hrp,
            tc.tile_pool(name="ps", bufs=3, space=bass.MemorySpace.PSUM) as ps,
            tc.tile_pool(name="zps", bufs=2, space=bass.MemorySpace.PSUM) as zps,
            tc.tile_pool(name="ops", bufs=1, space=bass.MemorySpace.PSUM) as opsp,
        ):
            nrmT_t = cpool.tile([3, NC], f32)
            nc.sync.dma_start(nrmT_t[:], nrmT[:])
            pc_t = cpool.tile([4, NC], f32)
            nc.sync.dma_start(pc_t[:], pc[:])
            wabc_t = cpool.tile([3, 3 * L * 8], f32)
            nc.sync.dma_start(wabc_t[:], wabc[:])
            wsig_t = cpool.tile([128, 512], f32)
            nc.sync.dma_start(wsig_t[:], wsig[:])
            wsum_t = cpool.tile([128, CHUNKS * L], f32)
            nc.sync.dma_start(wsum_t[:], wsum[:])
            cb_t = cpool.tile([128, 4], f32)
            nc.sync.dma_start(cb_t[:], cb[:])

            out_ps = opsp.tile([128, NC], f32)

            for c in range(CHUNKS):
                r_t = io.tile([128, NC], f32, tag="r")
                nc.sync.dma_start(r_t[:], rt[c * 128:(c + 1) * 128, :])
                wct_t = wpool.tile([128, LPC * 128], f32, tag="wct")
                nc.sync.dma_start(wct_t[:], wct[:, c * LPC * 128:(c + 1) * LPC * 128])
                wst_t = wpool.tile([128, LPC * 128], f32, tag="wst")
                nc.sync.dma_start(wst_t[:], wst[:, c * LPC * 128:(c + 1) * LPC * 128])
                wpc_t = wpool.tile([4, LPC * 128], f32, tag="wpc")
                nc.sync.dma_start(wpc_t[:], wpc[:, c * LPC * 128:(c + 1) * LPC * 128])

                ct_t = trig.tile([128, NC], f32, tag="ct")
                st_t = trig.tile([128, NC], f32, tag="st")
                # theta = (r + s)*pi/4 ; cos via sin(x + pi/2)
                nc.scalar.activation(ct_t[:], r_t[:], AF.Sin,
                                     bias=cb_t[:, 0:1], scale=PI4)
                nc.scalar.activation(st_t[:], r_t[:], AF.Sin,
                                     bias=cb_t[:, 1:2], scale=PI4)

                for hf in range(2):
                    fs = hf * HF
                    # hemisphere-mask dot products a,b,c -> cos_term > TINY
                    pa = ps.tile([128, HF], f32, tag="ph")
                    pb = ps.tile([128, HF], f32, tag="ph")
                    pcx = ps.tile([128, HF], f32, tag="ph")
                    nc.tensor.matmul(pa[:], wabc_t[:, c * 128:(c + 1) * 128],
                                     nrmT_t[:, fs:fs + HF], start=True, stop=True)
                    nc.tensor.matmul(pb[:], wabc_t[:, 1024 + c * 128:1024 + (c + 1) * 128],
                                     nrmT_t[:, fs:fs + HF], start=True, stop=True)
                    nc.tensor.matmul(pcx[:], wabc_t[:, 2048 + c * 128:2048 + (c + 1) * 128],
                                     nrmT_t[:, fs:fs + HF], start=True, stop=True)
                    q1 = work.tile([128, HF], f32, tag="q1")
                    q2 = work.tile([128, HF], f32, tag="q2")
                    q3 = work.tile([128, HF], f32, tag="q3")
                    msk = work.tile([128, HF], f32, tag="msk")
                    nc.vector.scalar_tensor_tensor(q1[:], ct_t[:, fs:fs + HF], 1.0,
                                                   pa[:], ALU.mult, ALU.mult)
                    nc.vector.scalar_tensor_tensor(q2[:], st_t[:, fs:fs + HF], 1.0,
                                                   pb[:], ALU.mult, ALU.mult)
                    nc.vector.tensor_add(q3[:], q1[:], q2[:])
                    nc.vector.scalar_tensor_tensor(q1[:], q3[:], 1.0,
                                                   pcx[:], ALU.mult, ALU.add)
                    nc.vector.tensor_scalar(msk[:], q1[:], TINY, 0.0,
                                            ALU.is_gt, ALU.bypass)

                    zt = zps.tile([128, HF], f32, tag="zt")
                    for j in range(LPC):
                        g, p = j // 8, j % 8
                        ph = ps.tile([128, HF], f32, tag="ph")
                        nc.tensor.matmul(ph[:], wct_t[:, j * 128:(j + 1) * 128],
                                         ct_t[:, fs:fs + HF],
                                         start=True, stop=False)
                        nc.tensor.matmul(ph[:], wst_t[:, j * 128:(j + 1) * 128],
                                         st_t[:, fs:fs + HF],
                                         start=False, stop=False)
                        nc.tensor.matmul(ph[:], wpc_t[:, j * 128:(j + 1) * 128],
                                         pc_t[:, fs:fs + HF],
                                         start=False, stop=True)
                        hr = hrp.tile([128, HF], f32, tag="hr")
                        nc.scalar.activation(hr[:], ph[:], AF.Relu,
                                             bias=cb_t[:, 3:4])
                        # z[(l',s'),n] for 8-lobe half-group g; block-diag
                        # lhsT column-block p selects this lobe's 8 columns.
                        nc.tensor.matmul(zt[g * 64:(g + 1) * 64, :],
                                         wsig_t[:, p * 64:(p + 1) * 64], hr[:],
                                         start=(p == 0), stop=(p == 7))
                    vis = work.tile([128, HF], f32, tag="vis")
                    nc.scalar.activation(vis[:], zt[:], AF.Sigmoid,
                                         bias=cb_t[:, 2:3])
                    vm = work.tile([128, HF], f32, tag="vm")
                    nc.vector.tensor_mul(vm[:], vis[:], msk[:])
                    # scale_l * sum_s: wsum is zero outside this chunk's 16
                    # columns; accumulate all chunks into the full-M out tile.
                    nc.tensor.matmul(out_ps[:, fs:fs + HF],
                                     wsum_t[:, c * L:(c + 1) * L], vm[:],
                                     start=(c == 0), stop=(c == CHUNKS - 1))

            out_sb = cpool.tile([128, NC], f32)
            nc.vector.tensor_copy(out_sb[:], out_ps[:])
            nc.sync.dma_start(out[:], out_sb[:])

    nc.compile()
    return nc


def _host_constants(points, normals, root_rot, lgtSGLobes, lgtSGLambdas,
                    W1, b1, W2, b2):
    f8 = np.float64
    lob = lgtSGLobes.astype(f8)
    l = lob / (np.linalg.norm(lob, axis=-1, keepdims=True) + TINY)
    z = np.zeros_like(l)
    z[:, 2] = 1.0
    U = np.cross(z, l)
    U = U / (np.linalg.norm(U, axis=-1, keepdims=True) + TINY)
    V = np.cross(l, U)
    V = V / (np.linalg.norm(V, axis=-1, keepdims=True) + TINY)
    sharp = lgtSGLambdas[:, 0].astype(f8)
    r_phi = np.minimum(np.arccos(1.0 - 1.0 / sharp), np.pi / 3.0)
    sp, cp = np.sin(r_phi), np.cos(r_phi)

    Wd = root_rot.astype(f8) @ W1[3:].astype(f8)          # [3,H]
    A = sp[:, None] * (U @ Wd)                             # [L,H]
    B = sp[:, None] * (V @ Wd)
    C = cp[:, None] * (l @ Wd)
    W1p = W1[:3].astype(f8)                                # [3,H]
    b1f = b1.astype(f8)
    w2 = W2[:, 0].astype(f8)
    w_l = np.exp(sharp * (cp - 1.0))
    scale_l = w_l / (S * w_l + TINY)
    spU = sp[:, None] * U
    spV = sp[:, None] * V
    cpl = cp[:, None] * l

    # wct/wst: [128, L*128]; col = l*128 + s*16 + h ;
    # row 8*(l%16)+s' = -A[l,h]*delta(s,s'), zero elsewhere (K=128 zero-pad)
    wctZ = np.zeros((128, L, 8, H), f8)
    wstZ = np.zeros((128, L, 8, H), f8)
    for ll in range(L):
        j = ll % LPC
        for s in range(8):
            wctZ[8 * j + s, ll, s, :] = -A[ll]
            wstZ[8 * j + s, ll, s, :] = -B[ll]
    wpc = np.zeros((4, L, 8, H), f8)
    wpc[0:3] = W1p[:, None, None, :]
    wpc[3] = (b1f[None, :] - C)[:, None, :]

    # wabc: [3, 3*1024]; block col = l*8+s
    wabc = np.zeros((3, 3, L, 8), f8)
    wabc[:, 0] = np.repeat(spU.T[:, :, None], 8, axis=2)
    wabc[:, 1] = np.repeat(spV.T[:, :, None], 8, axis=2)
    wabc[:, 2] = np.repeat(cpl.T[:, :, None], 8, axis=2)

    # wsig: [128, 8*64]; for in-group position p: cols p*64 + l''*8 + s' =
    # w2[h]*delta(s,s')*delta(l'',p)
    wsig = np.zeros((8, H, 8, 8, 8), f8)
    for p in range(8):
        for s in range(8):
            wsig[s, :, p, p, s] = w2
    # wsum: per-chunk [128, L] blocks; block cc maps chunk-local lobe lp to
    # global output column cc*16+lp (zero elsewhere).
    wsum = np.zeros((LPC, 8, CHUNKS, L), f8)
    for cc in range(CHUNKS):
        for lp in range(LPC):
            wsum[lp, :, cc, cc * LPC + lp] = scale_l[cc * LPC + lp]

    cbias = np.zeros((128, 4), f8)
    s_of_p = np.arange(128) % 8
    # ACT Sin LUT domain is [-pi, pi]; input is r*pi/4 + bias with r in [0,1),
    # so shift each s-row by a full period where needed to stay in range.
    cbias[:, 0] = (s_of_p * (np.pi / 4.0) + np.pi / 2.0
                   - 2.0 * np.pi * (s_of_p >= 2))         # cos bias
    cbias[:, 1] = (s_of_p * (np.pi / 4.0)
                   - 2.0 * np.pi * (s_of_p >= 4))         # sin bias
    cbias[:, 2] = float(b2[0])                            # sigmoid bias
    cbias[:, 3] = 0.0                                     # relu bias

    f32 = np.float32
    return dict(
        wct=np.ascontiguousarray(wctZ.reshape(128, L * 128), f32),
        wst=np.ascontiguousarray(wstZ.reshape(128, L * 128), f32),
        wpc=np.ascontiguousarray(wpc.reshape(4, L * 128), f32),
        wabc=np.ascontiguousarray(wabc.reshape(3, 3 * L * 8), f32),
        wsig=np.ascontiguousarray(wsig.reshape(128, 512), f32),
        wsum=np.ascontiguousarray(wsum.reshape(128, CHUNKS * L), f32),
        cb=np.ascontiguousarray(cbias, f32),
    )


def _make_in_maps(inputs):
    const = _host_constants(inputs["points"], inputs["normals"],
                            inputs["root_rot"], inputs["lgtSGLobes"],
                            inputs["lgtSGLambdas"], inputs["W1"],
                            inputs["b1"], inputs["W2"], inputs["b2"])
    f32 = np.float32
    r_t = np.asarray(inputs["r_theta_random"], f32).transpose(1, 2, 0).reshape(L * S, N)
    pT = np.asarray(inputs["points"], f32).T
    nT = np.asarray(inputs["normals"], f32).T
    ones = np.ones((1, N), f32)
    in_maps = []
    for c in range(NCORES):
        sl = slice(c * NC, (c + 1) * NC)
        m = dict(const)
        m["rt"] = np.ascontiguousarray(r_t[:, sl])
        m["nrmT"] = np.ascontiguousarray(nT[:, sl])
        m["pc"] = np.ascontiguousarray(
            np.concatenate([pT[:, sl], ones[:, sl]], axis=0))
        in_maps.append(m)
    return in_maps


def kernel(points, normals, root_rot, lgtSGLobes, lgtSGLambdas,
           r_theta_random, W1, b1, W2, b2):
    global _PROG
    from concourse.bass_utils import run_bass_kernel_spmd

    if _PROG is None:
        _PROG = _build_program()
    nc = _PROG

    in_maps = _make_in_maps(dict(
        points=points, normals=normals, root_rot=root_rot,
        lgtSGLobes=lgtSGLobes, lgtSGLambdas=lgtSGLambdas,
        r_theta_random=r_theta_random, W1=W1, b1=b1, W2=W2, b2=b2))

    res = run_bass_kernel_spmd(nc, in_maps, list(range(NCORES)))

    f32 = np.float32

    out_full = np.empty((N, L), f32)
    for c in range(NCORES):
        out_full[c * NC:(c + 1) * NC, :] = res.results[c]["out"].T
    return out_full


# revision 15
# speedup vs baseline: 1.0644x; 1.0644x over previous
"""Trainium2 Bass kernel for the SG-visibility sampling network.

Math notes (exploited structure):
  - U,V are orthogonal to the unit lobe axis l, so dot(sample_dir, l) == cos(r_phi)
    exactly (up to fp eps).  Hence the SG weight w = exp(sharp*(cos_phi-1)) is a
    per-lobe constant and sum_s(vis*w)/(sum_s w + TINY) = scale_l * sum_s vis with
    scale_l = w/(S*w + TINY), precomputed on host.
  - pre-activation of the hidden layer decomposes as
        pre_h[n,l,s,h] = P_n[h] - C_l[h] - ct[n,l,s]*A_l[h] - st[n,l,s]*B_l[h]
    with P_n = p_n @ W1[:3] + b1,  A_l = sp_l*(U_l@Wd),  B_l = sp_l*(V_l@Wd),
    C_l = cp_l*(l_l@Wd),  Wd = root_rot @ W1[3:].
  - hemisphere mask: cos_term = ct*a_nl + st*b_nl + c_nl with
    a = normals@(sp*U)_l, b = normals@(sp*V)_l, c = normals@(cp*l)_l.

Device layout (per core, data-parallel over N):  partitions = (lobe-in-chunk 16,
s 8) = 128, free dim = rays n (1024).  Per (lobe, n-half) the 8x16=(s,h) hidden
pre-activation lives in PSUM as (s,h) x n, built from 3 accumulating matmuls
with zero-padded K=128 stationary weights (PE base-partition rule: operands
must start at partition 0/32/64, so per-lobe K-slices are packed into lhsT
zeros instead).
"""

import numpy as np

N, L, S, H = 8192, 128, 8, 16
NCORES = 8
NC = N // NCORES          # rays per core
LPC = 16                  # lobes per chunk
CHUNKS = L // LPC
TINY = 1e-6

_PROG = None


def _build_program():
    import concourse.bass as bass
    import concourse.bacc as bacc
    import concourse.mybir as mybir
    import concourse.tile as tile

    f32 = mybir.dt.float32
    f32r = mybir.dt.float32r
    R = lambda ap: ap.bitcast(f32r)
    AF = mybir.ActivationFunctionType
    ALU = mybir.AluOpType
    PI4 = float(np.pi / 4.0)

    nc = bacc.Bacc("TRN2", target_bir_lowering=False, debug=False,
                   num_devices=NCORES)

    rt = nc.declare_dram_parameter("rt", [L * S, NC], f32, isOutput=False)
    nrmT = nc.declare_dram_parameter("nrmT", [3, NC], f32, isOutput=False)
    pc = nc.declare_dram_parameter("pc", [4, NC], f32, isOutput=False)
    wct = nc.declare_dram_parameter("wct", [128, L * 128], f32, isOutput=False)
    wst = nc.declare_dram_parameter("wst", [128, L * 128], f32, isOutput=False)
    wpc = nc.declare_dram_parameter("wpc", [4, L * 128], f32, isOutput=False)
    wabc = nc.declare_dram_parameter("wabc", [3, 3 * L * 8], f32, isOutput=False)
    wsig = nc.declare_dram_parameter("wsig", [128, 512], f32, isOutput=False)
    wsum = nc.declare_dram_parameter("wsum", [128, CHUNKS * L], f32, isOutput=False)
    cb = nc.declare_dram_parameter("cb", [128, 4], f32, isOutput=False)
    out = nc.declare_dram_parameter("out", [L, NC], f32, isOutput=True)

    HF = NC // 2  # matmul moving-operand free-dim limit for fp32

    with tile.TileContext(nc) as tc:
        with (
            tc.tile_pool(name="const", bufs=1) as cpool,
            tc.tile_pool(name="io", bufs=2) as io,
            tc.tile_pool(name="wp", bufs=2) as wpool,
            tc.tile_pool(name="trig", bufs=2) as trig,
            tc.tile_pool(name="work", bufs=2) as work,
            tc.tile_pool(name="hrp", bufs=3) as hrp,
            tc.tile_pool(name="ps", bufs=3, space=bass.MemorySpace.PSUM) as ps,
            tc.tile_pool(name="zps", bufs=1, space=bass.MemorySpace.PSUM) as zps,
            tc.tile_pool(name="ops", bufs=1, space=bass.MemorySpace.PSUM) as opsp,
        ):
            nrmT_t = cpool.tile([3, NC], f32)
            nc.sync.dma_start(nrmT_t[:], nrmT[:])
            pc_t = cpool.tile([4, NC], f32)
            nc.sync.dma_start(pc_t[:], pc[:])
            # f32r copy (PE consumes rounded f32r moving operands)
            pc_r = cpool.tile([4, NC], f32r)
            nc.vector.tensor_copy(pc_r[:], pc_t[:])
            wabc_t = cpool.tile([3, 3 * L * 8], f32)
            nc.sync.dma_start(wabc_t[:], wabc[:])
            wsig_t = cpool.tile([128, 512], f32)
            nc.sync.dma_start(wsig_t[:], wsig[:])
            wsum_t = cpool.tile([128, CHUNKS * L], f32)
            nc.sync.dma_start(wsum_t[:], wsum[:])
            wsig_r = cpool.tile([128, 512], f32r)
            nc.vector.tensor_copy(wsig_r[:], wsig_t[:])
            wsum_r = cpool.tile([128, CHUNKS * L], f32r)
            nc.vector.tensor_copy(wsum_r[:], wsum_t[:])
            cb_t = cpool.tile([128, 4], f32)
            nc.sync.dma_start(cb_t[:], cb[:])

            out_ps = opsp.tile([128, NC], f32)

            for c in range(CHUNKS):
                r_t = io.tile([128, NC], f32, tag="r")
                nc.sync.dma_start(r_t[:], rt[c * 128:(c + 1) * 128, :])
                wct_t = wpool.tile([128, LPC * 128], f32, tag="wct")
                nc.sync.dma_start(wct_t[:], wct[:, c * LPC * 128:(c + 1) * LPC * 128])
                wst_t = wpool.tile([128, LPC * 128], f32, tag="wst")
                nc.sync.dma_start(wst_t[:], wst[:, c * LPC * 128:(c + 1) * LPC * 128])
                wpc_t = wpool.tile([4, LPC * 128], f32, tag="wpc")
                nc.sync.dma_start(wpc_t[:], wpc[:, c * LPC * 128:(c + 1) * LPC * 128])
                wct_r = wpool.tile([128, LPC * 128], f32r, tag="wctr")
                nc.vector.tensor_copy(wct_r[:], wct_t[:])
                wst_r = wpool.tile([128, LPC * 128], f32r, tag="wstr")
                nc.vector.tensor_copy(wst_r[:], wst_t[:])
                wpc_r = wpool.tile([4, LPC * 128], f32r, tag="wpcr")
                nc.vector.tensor_copy(wpc_r[:], wpc_t[:])

                ct_t = trig.tile([128, NC], f32, tag="ct")
                st_t = trig.tile([128, NC], f32, tag="st")
                # theta = (r + s)*pi/4 ; cos via sin(x + pi/2)
                nc.scalar.activation(ct_t[:], r_t[:], AF.Sin,
                                     bias=cb_t[:, 0:1], scale=PI4)
                nc.scalar.activation(st_t[:], r_t[:], AF.Sin,
                                     bias=cb_t[:, 1:2], scale=PI4)
                # f32r twins for the PE moving operand (rounded by DVE copy);
                # the mask path needs full-fp32 ct/st to match reference signs.
                ct_r = trig.tile([128, NC], f32r, tag="ctr")
                st_r = trig.tile([128, NC], f32r, tag="str")
                nc.vector.tensor_copy(ct_r[:], ct_t[:])
                nc.vector.tensor_copy(st_r[:], st_t[:])

                for hf in range(2):
                    fs = hf * HF
                    # hemisphere-mask dot products a,b,c -> cos_term > TINY
                    pa = ps.tile([128, HF], f32, tag="ph")
                    pb = ps.tile([128, HF], f32, tag="ph")
                    pcx = ps.tile([128, HF], f32, tag="ph")
                    nc.tensor.matmul(pa[:], wabc_t[:, c * 128:(c + 1) * 128],
                                     nrmT_t[:, fs:fs + HF], start=True, stop=True)
                    nc.tensor.matmul(pb[:], wabc_t[:, 1024 + c * 128:1024 + (c + 1) * 128],
                                     nrmT_t[:, fs:fs + HF], start=True, stop=True)
                    nc.tensor.matmul(pcx[:], wabc_t[:, 2048 + c * 128:2048 + (c + 1) * 128],
                                     nrmT_t[:, fs:fs + HF], start=True, stop=True)
                    q1 = work.tile([128, HF], f32, tag="q1")
                    q2 = work.tile([128, HF], f32, tag="q2")
                    q3 = work.tile([128, HF], f32, tag="q3")
                    msk = work.tile([128, HF], f32, tag="msk")
                    nc.vector.scalar_tensor_tensor(q1[:], ct_t[:, fs:fs + HF], 1.0,
                                                   pa[:], ALU.mult, ALU.mult)
                    nc.vector.scalar_tensor_tensor(q2[:], st_t[:, fs:fs + HF], 1.0,
                                                   pb[:], ALU.mult, ALU.mult)
                    nc.vector.tensor_add(q3[:], q1[:], q2[:])
                    nc.vector.scalar_tensor_tensor(q1[:], q3[:], 1.0,
                                                   pcx[:], ALU.mult, ALU.add)
                    nc.vector.tensor_scalar(msk[:], q1[:], TINY, 0.0,
                                            ALU.is_gt, ALU.bypass)

                    zt0 = zps.tile([64, HF], f32, tag="zt0")
                    zt1 = zps.tile([64, HF], f32, tag="zt1")
                    zts = (zt0, zt1)
                    for j in range(LPC):
                        g, p = j // 8, j % 8
                        ph = ps.tile([128, HF], f32, tag="ph")
                        nc.tensor.matmul(ph[:], wct_r[:, j * 128:(j + 1) * 128],
                                         ct_r[:, fs:fs + HF],
                                         start=True, stop=False)
                        nc.tensor.matmul(ph[:], wst_r[:, j * 128:(j + 1) * 128],
                                         st_r[:, fs:fs + HF],
                                         start=False, stop=False)
                        nc.tensor.matmul(ph[:], wpc_r[:, j * 128:(j + 1) * 128],
                                         pc_r[:, fs:fs + HF],
                                         start=False, stop=True)
                        hr = hrp.tile([128, HF], f32r, tag="hr")
                        nc.scalar.activation(hr[:], ph[:], AF.Relu,
                                             bias=cb_t[:, 3:4])
                        # z[(l',s'),n] for 8-lobe half-group g; block-diag
                        # lhsT column-block p selects this lobe's 8 columns.
                        nc.tensor.matmul(zts[g][:, :],
                                         wsig_r[:, p * 64:(p + 1) * 64], hr[:],
                                         start=(p == 0), stop=(p == 7))
                    vis = work.tile([128, HF], f32, tag="vis")
                    nc.scalar.activation(vis[0:64, :], zt0[:], AF.Sigmoid,
                                         bias=cb_t[0:64, 2:3])
                    nc.scalar.activation(vis[64:128, :], zt1[:], AF.Sigmoid,
                                         bias=cb_t[64:128, 2:3])
                    vm = work.tile([128, HF], f32r, tag="vm")
                    nc.vector.tensor_mul(vm[:], vis[:], msk[:])
                    # scale_l * sum_s: wsum is zero outside this chunk's 16
                    # columns; accumulate all chunks into the full-M out tile.
                    nc.tensor.matmul(out_ps[:, fs:fs + HF],
                                     wsum_r[:, c * L:(c + 1) * L], vm[:],
                                     start=(c == 0), stop=(c == CHUNKS - 1))

            out_sb = cpool.tile([128, NC], f32)
            nc.vector.tensor_copy(out_sb[:], out_ps[:])
            nc.sync.dma_start(out[:], out_sb[:])

    nc.compile()
    return nc


def _host_constants(points, normals, root_rot, lgtSGLobes, lgtSGLambdas,
                    W1, b1, W2, b2):
    f8 = np.float64
    lob = lgtSGLobes.astype(f8)
    l = lob / (np.linalg.norm(lob, axis=-1, keepdims=True) + TINY)
    z = np.zeros_like(l)
    z[:, 2] = 1.0
    U = np.cross(z, l)
    U = U / (np.linalg.norm(U, axis=-1, keepdims=True) + TINY)
    V = np.cross(l, U)
    V = V / (np.linalg.norm(V, axis=-1, keepdims=True) + TINY)
    sharp = lgtSGLambdas[:, 0].astype(f8)
    r_phi = np.minimum(np.arccos(1.0 - 1.0 / sharp), np.pi / 3.0)
    sp, cp = np.sin(r_phi), np.cos(r_phi)

    Wd = root_rot.astype(f8) @ W1[3:].astype(f8)          # [3,H]
    A = sp[:, None] * (U @ Wd)                             # [L,H]
    B = sp[:, None] * (V @ Wd)
    C = cp[:, None] * (l @ Wd)
    W1p = W1[:3].astype(f8)                                # [3,H]
    b1f = b1.astype(f8)
    w2 = W2[:, 0].astype(f8)
    w_l = np.exp(sharp * (cp - 1.0))
    scale_l = w_l / (S * w_l + TINY)
    spU = sp[:, None] * U
    spV = sp[:, None] * V
    cpl = cp[:, None] * l

    # wct/wst: [128, L*128]; col = l*128 + s*16 + h ;
    # row 8*(l%16)+s' = -A[l,h]*delta(s,s'), zero elsewhere (K=128 zero-pad)
    wctZ = np.zeros((128, L, 8, H), f8)
    wstZ = np.zeros((128, L, 8, H), f8)
    for ll in range(L):
        j = ll % LPC
        for s in range(8):
            wctZ[8 * j + s, ll, s, :] = -A[ll]
            wstZ[8 * j + s, ll, s, :] = -B[ll]
    wpc = np.zeros((4, L, 8, H), f8)
    wpc[0:3] = W1p[:, None, None, :]
    wpc[3] = (b1f[None, :] - C)[:, None, :]

    # wabc: [3, 3*1024]; block col = l*8+s
    wabc = np.zeros((3, 3, L, 8), f8)
    wabc[:, 0] = np.repeat(spU.T[:, :, None], 8, axis=2)
    wabc[:, 1] = np.repeat(spV.T[:, :, None], 8, axis=2)
    wabc[:, 2] = np.repeat(cpl.T[:, :, None], 8, axis=2)

    # wsig: [128, 8*64]; for in-group position p: cols p*64 + l''*8 + s' =
    # w2[h]*delta(s,s')*delta(l'',p)
    wsig = np.zeros((8, H, 8, 8, 8), f8)
    for p in range(8):
        for s in range(8):
            wsig[s, :, p, p, s] = w2
    # wsum: per-chunk [128, L] blocks; block cc maps chunk-local lobe lp to
    # global output column cc*16+lp (zero elsewhere).
    wsum = np.zeros((LPC, 8, CHUNKS, L), f8)
    for cc in range(CHUNKS):
        for lp in range(LPC):
            wsum[lp, :, cc, cc * LPC + lp] = scale_l[cc * LPC + lp]

    cbias = np.zeros((128, 4), f8)
    s_of_p = np.arange(128) % 8
    # ACT Sin LUT domain is [-pi, pi]; input is r*pi/4 + bias with r in [0,1),
    # so shift each s-row by a full period where needed to stay in range.
    cbias[:, 0] = (s_of_p * (np.pi / 4.0) + np.pi / 2.0
                   - 2.0 * np.pi * (s_of_p >= 2))         # cos bias
    cbias[:, 1] = (s_of_p * (np.pi / 4.0)
                   - 2.0 * np.pi * (s_of_p >= 4))         # sin bias
    cbias[:, 2] = float(b2[0])                            # sigmoid bias
    cbias[:, 3] = 0.0                                     # relu bias

    f32 = np.float32
    return dict(
        wct=np.ascontiguousarray(wctZ.reshape(128, L * 128), f32),
        wst=np.ascontiguousarray(wstZ.reshape(128, L * 128), f32),
        wpc=np.ascontiguousarray(wpc.reshape(4, L * 128), f32),
        wabc=np.ascontiguousarray(wabc.reshape(3, 3 * L * 8), f32),
        wsig=np.ascontiguousarray(wsig.reshape(128, 512), f32),
        wsum=np.ascontiguousarray(wsum.reshape(128, CHUNKS * L), f32),
        cb=np.ascontiguousarray(cbias, f32),
    )


def _make_in_maps(inputs):
    const = _host_constants(inputs["points"], inputs["normals"],
                            inputs["root_rot"], inputs["lgtSGLobes"],
                            inputs["lgtSGLambdas"], inputs["W1"],
                            inputs["b1"], inputs["W2"], inputs["b2"])
    f32 = np.float32
    r_t = np.asarray(inputs["r_theta_random"], f32).transpose(1, 2, 0).reshape(L * S, N)
    pT = np.asarray(inputs["points"], f32).T
    nT = np.asarray(inputs["normals"], f32).T
    ones = np.ones((1, N), f32)
    in_maps = []
    for c in range(NCORES):
        sl = slice(c * NC, (c + 1) * NC)
        m = dict(const)
        m["rt"] = np.ascontiguousarray(r_t[:, sl])
        m["nrmT"] = np.ascontiguousarray(nT[:, sl])
        m["pc"] = np.ascontiguousarray(
            np.concatenate([pT[:, sl], ones[:, sl]], axis=0))
        in_maps.append(m)
    return in_maps


def kernel(points, normals, root_rot, lgtSGLobes, lgtSGLambdas,
           r_theta_random, W1, b1, W2, b2):
    global _PROG
    from concourse.bass_utils import run_bass_kernel_spmd

    if _PROG is None:
        _PROG = _build_program()
    nc = _PROG

    in_maps = _make_in_maps(dict(
        points=points, normals=normals, root_rot=root_rot,
        lgtSGLobes=lgtSGLobes, lgtSGLambdas=lgtSGLambdas,
        r_theta_random=r_theta_random, W1=W1, b1=b1, W2=W2, b2=b2))

    res = run_bass_kernel_spmd(nc, in_maps, list(range(NCORES)))

    f32 = np.float32

    out_full = np.empty((N, L), f32)
    for c in range(NCORES):
        out_full[c * NC:(c + 1) * NC, :] = res.results[c]["out"].T
    return out_full
